# revision 73
# baseline (speedup 1.0000x reference)
"""Trainium2 Bass kernel for nn_Block (LN -> local MHA -> LN -> global MHA -> LN -> MLP).

Sharding: pure data parallel, batch 8 across 8 cores (one batch element per
core), no collectives. All compute is feature-major ([D, S] transposed).

v4: on top of the fp8e4 DoubleRow / ALPHA-scaled bf16 residual design (see
scale ladder below), the schedule is tuned against the TimelineSim cost
model (297.4us -> 282.8us):

  - act-table hygiene: an explicit LoadActFuncSet(6) pins
    natural_log_exp_and_others (exp+ln+copy) at kernel start and rstd is
    computed as exp(-0.5*ln(var+eps)) on the Act engine, so the only act
    table switch left is the one into the Gelu set at the MLP tail
    (5 loads total vs 21).
  - the local-attention AV/den psum moved to its own bank pair ("avden"),
    decoupling the scores ring (PE->exp) from the normalize ring
    (AV -> recip/mul on DVE); local wave cadence no longer carries the
    DVE normalize latency.
  - out-proj residuals fold x into the psum via an identity matmul, making
    the writeback a pure drain that rotates across Act/DVE per phase
    (DRAIN_SEQ), like the qk/v projection drains; the xc8 quantize
    (SBUF-only) rotates across DVE/GPSIMD per LN layer (XC8_ENG). GPSIMD
    has no PSUM port, so only SBUF->SBUF sites may use it.
  - qkT/vnat are shared between the local and global layers (the global
    projections overwrite each s-block region only after the last
    local-attention read), halving their SBUF footprint.
  - input DMA issue order matches consumption order, and the first x
    s-block transfers in dt halves so LN1 stats start earlier.

Scale ladder (unchanged from v3):
  residual x' = ALPHA * x           (bf16; LN scale-invariant w/ eps' = eps*ALPHA^2)
  wq' = S_Q*Wq_eff, wk' = S_K*Wk, wv' = S_V*Wv  (fp8; xc8 = LN(x) true scale)
  scores psum = S_Q*S_K * s_true    -> exp(scale=1/(S_Q*S_K)) -> pt fp8 (true)
  V drains: v8 = S_V * v_true; den-ones = S_V/2 -> attnT = 2*attn_true (fp8)
  wo' = S_O*Wo with 2*S_O = ALPHA   -> out-proj psum = ALPHA*(Wo@attn)
  fc1 psum = S_1*h -> Gelu(scale=1/S_1) -> gT fp8 true; w2' = ALPHA*W2
"""

import math
import os
from contextlib import ExitStack

import numpy as np

import concourse.bacc as bacc
import concourse.bass as bass
import concourse.mybir as mybir
import concourse.tile as tile
from concourse import bass_utils

F32 = mybir.dt.float32
BF16 = mybir.dt.bfloat16
F8 = mybir.dt.float8e4
AF = mybir.ActivationFunctionType
ALU = mybir.AluOpType
DR = mybir.MatmulPerfMode.DoubleRow

NH = 4
BAND = 6
D = 512
B, S = 8, 2048
HD = 128
DT = D // 128
ET2 = (2 * D) // 128
SB = S // 512
ST = S // 128
EPS = 1e-5

ALPHA = 128.0
S_Q = 512.0
S_K = 64.0
S_V = 64.0
S_O = 64.0             # 2*S_O == ALPHA (attnT carries 2*attn via den-ones=S_V/2)
S_1 = 64.0
S_2 = ALPHA
EPS_EFF = EPS * ALPHA * ALPHA

_PHASE = {"n": 0}
MARKS = []


def _mark(nc, label):
    MARKS.append((label, nc.get_next_instruction_name()))


def _on():
    _PHASE["n"] += 1
    return _PHASE["n"] <= int(os.environ.get("K_STOP", "99"))


# Engine assignment for tunable elementwise sites: "v" = DVE, "g" = GPSIMD/Pool
# (GPSIMD has no PSUM port: only SBUF->SBUF sites may use "g".)
ENG = {
    "m2": "g",
    "unscale": "v",
}

# Per-dt engine for the xc8 quantize (SBUF->SBUF), keyed by LN layer.
XC8_ENG = {
    1: ("v", "g", "g", "g"),
    2: ("v", "g", "g", "v"),
    3: ("v", "v", "g", "g"),
}
# fc2 residual: accumulate x into the psum on PE (identity matmul), then
# drain on the otherwise-idle tail Act engine.


# PSUM->SBUF drain engine rotation per site ("a"=Act, "v"=DVE).
DRAIN_SEQ = {
    "qk_l": ("a", "a", "v"),
    "qk_g": ("a", "a", "v"),
    "v_l": ("a", "v"),
    "v_g": ("a", "v"),
    "op_l": ("a", "a", "v"),
    "op_g": ("v",),
    "fc2": ("v", "a"),
}
_DRAIN_CTR = {}


def _eng(nc, key):
    return nc.gpsimd if ENG[key] == "g" else nc.vector


def _drain(nc, dst, src_ap, site):
    seq = DRAIN_SEQ.get(site, ("v",))
    c = _DRAIN_CTR.get(site, 0)
    _DRAIN_CTR[site] = c + 1
    e = seq[c % len(seq)]
    if e == "a":
        nc.scalar.activation(dst, src_ap, AF.Copy)
    else:
        nc.vector.tensor_copy(dst, src_ap)


def build(use_op_bias=False, use_qkv_bias=False, b1_nonzero=False):
    _PHASE["n"] = 0
    MARKS.clear()
    _DRAIN_CTR.clear()
    nc = bacc.Bacc(trn_type="TRN2", target_bir_lowering=False, debug=False)
    drams = {}

    def din(name, shape, dtype, kind="ExternalInput"):
        drams[name] = nc.dram_tensor(name, shape, dtype, kind=kind)

    din("xTbf", [D, S], BF16)
    din("wqkvT8_l", [D, 3 * D], F8)
    din("wqkvT8_g", [D, 3 * D], F8)
    din("bqk_l_r1", [1, 2 * D], BF16)
    din("bqk_g_r1", [1, 2 * D], BF16)
    din("bv_l_r1", [1, D], BF16)
    din("bv_g_r1", [1, D], BF16)
    din("woT8_l", [D, D], F8)
    din("woT8_g", [D, D], F8)
    din("bo_l_r1", [1, D], BF16)
    din("bo_g_r1", [1, D], BF16)
    din("w1T8", [D, 2 * D], F8)
    din("b1", [2 * D], F32)
    din("w2T8", [2 * D, D], F8)
    din("b2_r1", [1, D], BF16)
    din("masksadd", [6, 128, 128], BF16)
    din("outT", [D, S], F32, kind="ExternalOutput")

    with tile.TileContext(nc) as tc:
        with ExitStack() as top:
            cpool = top.enter_context(tc.tile_pool(name="consts", bufs=1))
            ones_bf = cpool.tile([128, 128], BF16, tag="ones")
            nc.vector.memset(ones_bf, 1.0)           # LN stats matmul
            onesd_bf = cpool.tile([128, 128], BF16, tag="onesd")
            nc.vector.memset(onesd_bf, S_V / 2.0)    # local den (bf16 pt)
            ones8_2 = cpool.tile([128, 2, 128], F8, tag="ones8")
            nc.vector.memset(ones8_2, S_V / 2.0)     # global den (fp8 DR)
            ones_row = cpool.tile([1, 512], BF16, tag="onesr")
            nc.vector.memset(ones_row, 1.0)
            ones_col = cpool.tile([1, 128], BF16, tag="onesc")
            nc.vector.memset(ones_col, 1.0)
            _li = mybir.InstLoadActFuncSet(
                name=nc.get_next_instruction_name(), ins=[], outs=[],
                act_func_set_id=6)
            nc.scalar.add_instruction(_li)
            from concourse.masks import make_identity
            ident_bf = cpool.tile([128, 128], BF16, tag="ident")
            make_identity(nc, ident_bf)
            hid = top.enter_context(tc.tile_pool(name="hid", bufs=1))
            x = hid.tile([128, DT, S], BF16, tag="x")
            xbf_d = drams["xTbf"].ap().rearrange("(dt p) s -> p dt s", p=128)
            masks_sb = cpool.tile([128, 6, 128], BF16, tag="masks")

            wpool = top.enter_context(tc.tile_pool(name="weights", bufs=1))
            w8 = {}
            wo8 = {}
            bo_sb = {}
            bqk_r1 = {}
            bv_r1 = {}
            for wh in ("l", "g"):
                w8[wh] = wpool.tile([128, DT, 12 * 128], F8,
                                    tag=f"wqkv_{wh}", name=f"wqkv_{wh}")
                wo8[wh] = wpool.tile([128, NH, DT * 128], F8,
                                     tag=f"wo_{wh}", name=f"wo_{wh}")
                bo_sb[wh] = wpool.tile([1, 512], BF16, tag=f"bo_{wh}",
                                       name=f"bo_{wh}")
                if use_qkv_bias:
                    bqk_r1[wh] = wpool.tile([1, 1024], BF16,
                                            tag=f"bqk_{wh}", name=f"bqk_{wh}")
                    bv_r1[wh] = wpool.tile([1, 512], BF16, tag=f"bv_{wh}",
                                           name=f"bv_{wh}")
                else:
                    bqk_r1[wh] = bv_r1[wh] = None
            w18 = wpool.tile([128, DT, ET2 * 128], F8, tag="w1")
            w28 = wpool.tile([128, ET2, DT * 128], F8, tag="w2")
            b1_sb = wpool.tile([128, ET2], F32, tag="b1")
            b2_sb = wpool.tile([1, 512], BF16, tag="b2")

            # DMA issue order = consumption order: x(0), local weights, masks
            # (wave 0), remaining x, then the global/MLP weights.
            nc.sync.dma_start(x[:, 0:2, 0:512], xbf_d[:, 0:2, 0:512])
            nc.sync.dma_start(x[:, 2:4, 0:512], xbf_d[:, 2:4, 0:512])
            nc.sync.dma_start(w8["l"], drams["wqkvT8_l"].ap().rearrange(
                "(dt p) e -> p dt e", p=128))
            nc.sync.dma_start(masks_sb,
                              drams["masksadd"].ap().rearrange("m p j -> p m j"))
            for sb in range(1, SB):
                ssl = slice(sb * 512, (sb + 1) * 512)
                nc.sync.dma_start(x[:, :, ssl], xbf_d[:, :, ssl])
            nc.sync.dma_start(wo8["l"], drams["woT8_l"].ap().rearrange(
                "(h p) d -> p h d", p=128))
            nc.sync.dma_start(bo_sb["l"], drams["bo_l_r1"].ap())
            nc.sync.dma_start(w8["g"], drams["wqkvT8_g"].ap().rearrange(
                "(dt p) e -> p dt e", p=128))
            nc.sync.dma_start(wo8["g"], drams["woT8_g"].ap().rearrange(
                "(h p) d -> p h d", p=128))
            nc.sync.dma_start(bo_sb["g"], drams["bo_g_r1"].ap())
            if use_qkv_bias:
                for wh in ("l", "g"):
                    nc.sync.dma_start(bqk_r1[wh], drams[f"bqk_{wh}_r1"].ap())
                    nc.sync.dma_start(bv_r1[wh], drams[f"bv_{wh}_r1"].ap())
            nc.sync.dma_start(w18, drams["w1T8"].ap().rearrange(
                "(dt p) e -> p dt e", p=128))
            nc.sync.dma_start(w28, drams["w2T8"].ap().rearrange(
                "(e p) d -> p e d", p=128))
            nc.sync.dma_start(b1_sb, drams["b1"].ap().rearrange(
                "(e p) -> p e", p=128))
            nc.sync.dma_start(b2_sb, drams["b2_r1"].ap())

            act = top.enter_context(tc.tile_pool(name="act", bufs=1))
            xc8 = act.tile([128, DT, S], F8, tag="xc8")       # shared all layers
            # qkT / vnat are shared between the local and global layers: the
            # global projections overwrite each s-block region only after the
            # last local-attention read of it (subtile deps order the writes).
            qkT_sh = act.tile([128, 2 * NH, S], BF16, tag="qkT", name="qkT")
            qkT = {"l": qkT_sh, "g": qkT_sh}
            vnat_sh = act.tile([128, ST + 1, 512], F8, tag="vnat",
                               name="vnat")
            vnat = {"l": vnat_sh, "g": vnat_sh}
            attnT = act.tile([128, NH, S], F8, tag="attnT")   # shared l/g

            sbw = top.enter_context(tc.tile_pool(name="sbw", bufs=1))
            psA_stack = ExitStack()
            P = {"psum": psA_stack.enter_context(
                tc.tile_pool(name="psumA", bufs=1, space="PSUM"))}
            PH = {"bufs": 3}

            def switch_psum():
                """Close the local-phase psum pool (ps2 x3 + avden) and open
                the global-phase pool (big x1 + ps2 x1 + avden x1)."""
                psA_stack.close()
                P["psum"] = top.enter_context(
                    tc.tile_pool(name="psumB", bufs=1, space="PSUM"))
                PH["bufs"] = 1

            outT_d = drams["outT"].ap().rearrange("(dt p) s -> p dt s", p=128)

            # ---------------- per-s-block emitters ----------------

            ln_state = {}
            vpe_batches = {}

            def ln_stats(sb, bid, slot, ptag="ps2", pbufs=None,
                         stat_act=True, sq_eng="v"):
                """LN stats of residual x for one s-block.  The var+eps lands
                in slot `slot` of batch tile `bid` so a whole batch can be
                rstd'ed by a single sqrt instruction later."""
                ssl = slice(sb * 512, (sb + 1) * 512)
                sq = sbw.tile([128, DT, 512], BF16, tag="sq", bufs=1)
                if sq_eng == "a":
                    nc.scalar.activation(sq, x[:, :, ssl], AF.Square)
                else:
                    se = nc.vector if sq_eng == "v" else nc.gpsimd
                    se.tensor_mul(sq[:, 0:2, :], x[:, 0:2, ssl],
                                  x[:, 0:2, ssl])
                    se.tensor_mul(sq[:, 2:4, :], x[:, 2:4, ssl],
                                  x[:, 2:4, ssl])
                ps = P["psum"].tile([128, 2, 512], F32, tag=ptag,
                                    bufs=pbufs or PH["bufs"])
                for dt in range(DT):
                    nc.tensor.matmul(ps[:, 0, :], ones_bf, x[:, dt, ssl],
                                     start=(dt == 0), stop=(dt == DT - 1))
                    nc.tensor.matmul(ps[:, 1, :], ones_bf, sq[:, dt, :],
                                     start=(dt == 0), stop=(dt == DT - 1))
                if bid not in vpe_batches:
                    vpeb_t = sbw.tile([128, 2, 512], BF16, tag="vpeb",
                                      bufs=2, name=f"vpeb_{bid}")
                    vpe_batches[bid] = vpeb_t
                vpe = vpe_batches[bid][:, slot, :]
                meanb = sbw.tile([128, 512], BF16, tag="meanb", bufs=2)
                m2 = sbw.tile([128, 512], BF16, tag="m2", bufs=1)
                xcb = sbw.tile([128, DT, 512], BF16, tag="xcb", bufs=4)
                if stat_act:
                    nc.scalar.activation(meanb, ps[:, 0, :], AF.Copy,
                                         scale=1.0 / D)
                    nc.scalar.activation(vpe, ps[:, 1, :], AF.Copy,
                                         scale=1.0 / D, bias=EPS_EFF)
                else:
                    nc.vector.tensor_scalar(meanb, ps[:, 0, :], 1.0 / D, None,
                                            ALU.mult)
                    nc.vector.tensor_scalar(vpe, ps[:, 1, :], 1.0 / D,
                                            EPS_EFF, ALU.mult, ALU.add)
                _eng(nc, "m2").tensor_mul(m2, meanb, meanb)
                nc.vector.tensor_sub(vpe, vpe, m2)
                for dt in range(DT):
                    nc.vector.tensor_sub(xcb[:, dt, :], x[:, dt, ssl], meanb)
                ln_state[sb] = [bid, slot, xcb, None]

            def ln_rstd(sbs, recip=True):
                """rstd = exp(-0.5*ln(var+eps)) on the Act engine.  Both Ln
                and Exp live in act-func-set 6 (natural_log_exp_and_others),
                the set explicitly loaded at kernel start, so no act-table
                reloads happen no matter how the scheduler interleaves."""
                bid = ln_state[sbs[0]][0]
                slots = [ln_state[sb][1] for sb in sbs]
                lo, hi = min(slots), max(slots) + 1
                vpeb = vpe_batches[bid]
                lnv = sbw.tile([128, 2, 512], F32, tag="lnv", bufs=2)
                nc.scalar.activation(lnv[:, lo:hi, :], vpeb[:, lo:hi, :],
                                     AF.Ln)
                rstdb = sbw.tile([128, 2, 512], BF16, tag="rstdb", bufs=2)
                nc.scalar.activation(rstdb[:, lo:hi, :], lnv[:, lo:hi, :],
                                     AF.Exp, scale=-0.5)
                for sb in sbs:
                    ln_state[sb][3] = rstdb

            def ln_apply(sb, layer):
                """xc8 = xcb * rstd for one s-block (SBUF only: DVE/Pool)."""
                ssl = slice(sb * 512, (sb + 1) * 512)
                bid, slot, xcb, rstdb = ln_state.pop(sb)
                rstd = rstdb[:, slot, :]
                engs = XC8_ENG[layer]
                for dt in range(DT):
                    e = nc.vector if engs[dt] == "v" else nc.gpsimd
                    e.tensor_mul(xc8[:, dt, ssl], xcb[:, dt, :], rstd)

            # local V chunk starts: shifted grid so each q-tile's band is
            # covered by two adjacent chunks (DoubleRow-able)
            VCH = [0] + [128 * j - 64 for j in range(1, ST)] + [S - 128]
            VCH_SB = [[j for j in range(ST + 1)
                       if VCH[j] + 128 <= 512 * (sb + 1)
                       and (sb == 0 or VCH[j] + 128 > 512 * sb)]
                      for sb in range(SB)]

            def proj_v(sb, wh, ptag="ps2", pbufs=3):
                """V in natural (k-major) layout; xc8 chunk stationary."""
                w = w8[wh]
                chunks = (VCH_SB[sb] if wh == "l"
                          else list(range(4 * sb, 4 * sb + 4)))
                starts = {j: (VCH[j] if wh == "l" else j * 128)
                          for j in chunks}
                for p0 in range(0, len(chunks), 2):
                    pair = chunks[p0:p0 + 2]
                    ps = P["psum"].tile([128, 2, 512], F32, tag=ptag,
                                    bufs=pbufs or PH["bufs"])
                    for i, j in enumerate(pair):
                        csl = slice(starts[j], starts[j] + 128)
                        for dtp in range(0, DT, 2):
                            nc.tensor.matmul(
                                ps[:, i, :], xc8[:, dtp:dtp + 2, csl],
                                w[:, dtp:dtp + 2, 1024:1536],
                                start=(dtp == 0),
                                stop=(dtp == DT - 2 and bv_r1[wh] is None),
                                perf_mode=DR)
                        if bv_r1[wh] is not None:
                            nc.tensor.matmul(ps[:, i, :], ones_col, bv_r1[wh],
                                             start=False, stop=True)
                    if len(pair) == 2:
                        _drain(nc, vnat[wh][:, pair[0]:pair[0] + 2, :], ps, f"v_{wh}")
                    else:
                        _drain(nc, vnat[wh][:, pair[0], :], ps[:, 0, :], f"v_{wh}")

            def proj_qk(sb, wh, ptag="ps2", pbufs=3):
                w = w8[wh]
                ssl = slice(sb * 512, (sb + 1) * 512)
                for et0 in (4, 6, 0, 2):  # k heads first, then q
                    ps = P["psum"].tile([128, 2, 512], F32, tag=ptag,
                                    bufs=pbufs or PH["bufs"])
                    for i in range(2):
                        et = et0 + i
                        for dtp in range(0, DT, 2):
                            nc.tensor.matmul(
                                ps[:, i, :],
                                w[:, dtp:dtp + 2, et * 128:(et + 1) * 128],
                                xc8[:, dtp:dtp + 2, ssl],
                                start=(dtp == 0),
                                stop=(dtp == DT - 2 and bqk_r1[wh] is None),
                                perf_mode=DR)
                        if bqk_r1[wh] is not None:
                            nc.tensor.matmul(
                                ps[:, i, :],
                                bqk_r1[wh][:1, et * 128:(et + 1) * 128],
                                ones_row, start=False, stop=True)
                    _drain(nc, qkT[wh][:, et0:et0 + 2, ssl], ps, f"qk_{wh}")

            _attn_state = {"pre": {}}

            def attn_pre(wh, qb, h, ktps):
                """Head-start: scores+exp only for the given kt pairs; the
                pt tiles are stashed and consumed by the resume pass."""
                qk = qkT[wh]
                qsl = slice(qb * 512, (qb + 1) * 512)
                for ktp in ktps:
                    ps = P["psum"].tile([128, 2, 512], F32, tag="ps2",
                                        bufs=PH["bufs"])
                    for i in range(2):
                        kt = ktp + i
                        nc.tensor.matmul(
                            ps[:, i, :],
                            qk[:, NH + h, kt * 128:(kt + 1) * 128],
                            qk[:, h, qsl], start=True, stop=True)
                    pt = sbw.tile([128, 2, 512], F8, tag="pt", bufs=16)
                    nc.scalar.activation(pt, ps, AF.Exp,
                                         scale=1.0 / (S_Q * S_K))
                    _attn_state["pre"][(wh, qb, h, ktp)] = pt

            # ktp pairs per (qb, h), grouped so a 4-bank psum tile gives
            # one 2048-wide exp for two pairs (phase-B pools only).
            ATTN_GROUPS = (("b", (0, 2)), ("s", (4,)), ("b", (6, 8)),
                           ("s", (10,)), ("b", (12, 14)))

            def attn_block(wh, qb, mid=None):
                qk = qkT[wh]
                vn = vnat[wh]
                pre = _attn_state["pre"]
                qsl = slice(qb * 512, (qb + 1) * 512)
                for h in range(NH):
                    if mid is not None and h in mid:
                        mid[h]()
                    popd = P["psum"].tile([128, 2, 512], F32, tag="avden",
                                          bufs=1)
                    ndone = [0]

                    def avden_pair(ktp, pt_ap):
                        ndone[0] += 1
                        nc.tensor.matmul(
                            popd[:, 0, :],
                            vn[:, ktp:ktp + 2, h * 128:(h + 1) * 128],
                            pt_ap, start=(ndone[0] == 1),
                            stop=(ndone[0] == ST // 2), perf_mode=DR)
                        nc.tensor.matmul(
                            popd[:, 1, :], ones8_2, pt_ap,
                            start=(ndone[0] == 1), stop=(ndone[0] == ST // 2),
                            perf_mode=DR)

                    def score_pair_into(ps_rows, k):
                        for j in range(2):
                            nc.tensor.matmul(
                                ps_rows[:, j, :],
                                qk[:, NH + h, (k + j) * 128:(k + j + 2) * 128
                                   - 128],
                                qk[:, h, qsl], start=True, stop=True)

                    for kind, ktps in ATTN_GROUPS:
                        stashed = [k for k in ktps
                                   if (wh, qb, h, k) in pre]
                        missing = [k for k in ktps if k not in stashed]
                        for k in stashed:
                            avden_pair(k, pre.pop((wh, qb, h, k)))
                        if not missing:
                            continue
                        if kind == "b" and len(missing) == 2:
                            ps = P["psum"].tile([128, 4, 512], F32,
                                                tag="big", bufs=1)
                            score_pair_into(ps[:, 0:2, :], missing[0])
                            score_pair_into(ps[:, 2:4, :], missing[1])
                            pt4 = sbw.tile([128, 4, 512], F8, tag="pt4",
                                           bufs=2)
                            nc.scalar.activation(pt4, ps, AF.Exp,
                                                 scale=1.0 / (S_Q * S_K))
                            avden_pair(missing[0], pt4[:, 0:2, :])
                            avden_pair(missing[1], pt4[:, 2:4, :])
                        else:
                            for k in missing:
                                ps = P["psum"].tile([128, 2, 512], F32,
                                                    tag="ps2",
                                                    bufs=PH["bufs"])
                                score_pair_into(ps, k)
                                pt = sbw.tile([128, 2, 512], F8, tag="pt",
                                              bufs=16)
                                nc.scalar.activation(
                                    pt, ps, AF.Exp, scale=1.0 / (S_Q * S_K))
                                avden_pair(k, pt)
                    rden = sbw.tile([128, 512], F32, tag="rden", bufs=2)
                    nc.vector.reciprocal(rden, popd[:, 1, :])
                    nc.vector.tensor_mul(attnT[:, h, qb * 512:(qb + 1) * 512],
                                         popd[:, 0, :], rden)

            def attn_local_factory():
                """Local attention, qt-major with all heads batched.
                Scores + additive band masks accumulate in one [128,4,2,128]
                PSUM quad; one exp per q-tile; fp8 DoubleRow AV/den on the
                shifted V grid; per-qt normalize. Returns step(w) emitting
                one skewed pipeline wave; call w = 0..ST+1."""
                qk = qkT["l"]
                vn = vnat["l"]
                sc = {}
                pts = {}
                pops = {}

                def emit_scores(qt):
                    ps = P["psum"].tile([128, NH, 2, 128], F32, tag="ps2",
                                        bufs=3)
                    sc[qt] = ps
                    # mask class: 0 first tile, 1 interior, 2 last
                    cls = 0 if qt == 0 else (2 if qt == ST - 1 else 1)
                    qsl = slice(qt * 128, (qt + 1) * 128)
                    for h in range(NH):
                        for i in range(2):
                            o = VCH[qt + i]
                            nc.tensor.matmul(
                                ps[:, h, i, :], qk[:, NH + h, o:o + 128],
                                qk[:, h, qsl], start=True, stop=False)
                            nc.tensor.matmul(
                                ps[:, h, i, :],
                                masks_sb[:, 2 * cls + i, :], ident_bf,
                                start=False, stop=True)

                def emit_exp(qt):
                    pt = sbw.tile([128, NH, 2, 128], F8, tag="ptl", bufs=3)
                    pts[qt] = pt
                    nc.scalar.activation(pt, sc[qt], AF.Exp,
                                         scale=1.0 / (S_Q * S_K))
                    del sc[qt]

                def emit_avden(qt):
                    popd = P["psum"].tile([128, 2, NH, 128], F32,
                                          tag="avden", bufs=1)
                    pops[qt] = popd
                    pt = pts[qt]
                    for h in range(NH):
                        nc.tensor.matmul(
                            popd[:, 0, h, :],
                            vn[:, qt:qt + 2, h * 128:(h + 1) * 128],
                            pt[:, h, :, :], start=True, stop=True,
                            perf_mode=DR)
                        nc.tensor.matmul(
                            popd[:, 1, h, :], ones8_2, pt[:, h, :, :],
                            start=True, stop=True, perf_mode=DR)
                    del pts[qt]

                def emit_norm(qt):
                    popd = pops.pop(qt)
                    qsl = slice(qt * 128, (qt + 1) * 128)
                    rden = sbw.tile([128, NH, 128], F32, tag="rden", bufs=2)
                    nc.vector.reciprocal(rden, popd[:, 1, :, :])
                    nc.vector.tensor_mul(attnT[:, :, qsl], popd[:, 0, :, :],
                                         rden)

                def step(w):
                    if w < ST:
                        emit_scores(w)
                    if 1 <= w <= ST:
                        emit_exp(w - 1)
                    if w >= 2:
                        emit_avden(w - 2)
                        emit_norm(w - 2)

                return step

            def op_block(wh, sb):
                """Out-proj + residual: x folded into the psum via an
                identity matmul; the writeback is then a pure drain that can
                rotate across engines."""
                ssl = slice(sb * 512, (sb + 1) * 512)
                for dtp in range(0, DT, 2):
                    ps = P["psum"].tile([128, 2, 512], F32, tag="ps2",
                                        bufs=PH["bufs"])
                    for i in range(2):
                        dt = dtp + i
                        for hp in range(0, NH, 2):
                            nc.tensor.matmul(
                                ps[:, i, :],
                                wo8[wh][:, hp:hp + 2, dt * 128:(dt + 1) * 128],
                                attnT[:, hp:hp + 2, ssl],
                                start=(hp == 0), stop=False,
                                perf_mode=DR)
                        if use_op_bias:
                            nc.tensor.matmul(
                                ps[:, i, :],
                                bo_sb[wh][:1, dt * 128:(dt + 1) * 128],
                                ones_row, start=False, stop=False)
                        nc.tensor.matmul(
                            ps[:, i, :], ident_bf, x[:, dt, ssl],
                            start=False, stop=True)
                    _drain(nc, x[:, dtp:dtp + 2, ssl], ps, f"op_{wh}")

            g_tiles = {}

            def mlp_fc1(sb):
                ssl = slice(sb * 512, (sb + 1) * 512)
                gT = sbw.tile([128, ET2, 512], F8, tag="gT", bufs=2)
                g_tiles[sb] = gT
                for e2q in range(0, ET2, 4):
                    ps = P["psum"].tile([128, 4, 512], F32, tag="big", bufs=1)
                    for i in range(4):
                        e2 = e2q + i
                        for dtp in range(0, DT, 2):
                            nc.tensor.matmul(
                                ps[:, i, :],
                                w18[:, dtp:dtp + 2, e2 * 128:(e2 + 1) * 128],
                                xc8[:, dtp:dtp + 2, ssl],
                                start=(dtp == 0), stop=(dtp == DT - 2),
                                perf_mode=DR)
                    if b1_nonzero:
                        for i in range(4):
                            nc.scalar.activation(
                                gT[:, e2q + i, :], ps[:, i, :], AF.Gelu,
                                bias=b1_sb[:, e2q + i:e2q + i + 1],
                                scale=1.0 / S_1)
                    else:
                        nc.scalar.activation(gT[:, e2q:e2q + 4, :], ps,
                                             AF.Gelu, scale=1.0 / S_1)

            def mlp_fc2(sb):
                ssl = slice(sb * 512, (sb + 1) * 512)
                gT = g_tiles.pop(sb)
                for dtp in range(0, DT, 2):
                    if dtp == 0:
                        ps = P["psum"].tile([128, 2, 512], F32, tag="ps2",
                                        bufs=PH["bufs"])
                    else:
                        ps = P["psum"].tile([128, 2, 512], F32, tag="ps2",
                                            bufs=PH["bufs"])
                    xo = sbw.tile([128, 2, 512], F32, tag="xout", bufs=2)
                    for i in range(2):
                        dt = dtp + i
                        for e2p in range(0, ET2, 2):
                            nc.tensor.matmul(
                                ps[:, i, :],
                                w28[:, e2p:e2p + 2, dt * 128:(dt + 1) * 128],
                                gT[:, e2p:e2p + 2, :],
                                start=(e2p == 0), stop=False,
                                perf_mode=DR)
                        if use_op_bias:
                            nc.tensor.matmul(
                                ps[:, i, :],
                                b2_sb[:1, dt * 128:(dt + 1) * 128],
                                ones_row, start=False, stop=False)
                        nc.tensor.matmul(
                            ps[:, i, :], ident_bf, x[:, dt, ssl],
                            start=False, stop=True)
                    _drain(nc, xo, ps, "fc2")
                    nc.sync.dma_start(outT_d[:, dtp:dtp + 2, ssl], xo)

            # ---------------- pipeline schedule ----------------

            if _on():
                step = attn_local_factory()
                def ln_inline(sb, bid, layer):
                    ln_stats(sb, bid, 0)
                    ln_rstd((sb,))
                    ln_apply(sb, layer)

                _mark(nc, 'prologue')
                ln_inline(0, "L10", 1)
                ln_inline(1, "L11", 1)
                ln_inline(2, "L12", 1)
                proj_qk(0, "l")
                proj_v(0, "l")
                _mark(nc, 'waves012')
                for w in (0, 1, 2):
                    step(w)
                ln_inline(3, "L13", 1)
                proj_qk(1, "l")
                proj_v(1, "l")
                for w in (3, 4, 5, 6):
                    step(w)
                op_block("l", 0)
                ln_inline(0, "L20", 2)
                proj_qk(2, "l")
                proj_v(2, "l")
                for w in (7, 8, 9, 10):
                    step(w)
                proj_qk(3, "l")
                proj_v(3, "l")
                op_block("l", 1)
                ln_inline(1, "L21", 2)
                proj_qk(0, "g")
                proj_v(0, "g")
                for w in (11, 12, 13):
                    step(w)
                op_block("l", 2)
                ln_inline(2, "L22", 2)
                proj_qk(1, "g")
                proj_v(1, "g")
                _mark(nc, 'waves14+')
                for w in (14, 15, 16, 17):
                    step(w)
                _mark(nc, 'op_l3')
                op_block("l", 3)
                ln_inline(3, "L23", 2)
                proj_qk(2, "g")
                proj_v(2, "g")
                _mark(nc, 'projg3')
                attn_pre("g", 0, 0, (0, 2, 4, 6))
                attn_pre("g", 0, 1, (0, 2))
                proj_qk(3, "g")
                proj_v(3, "g")
                switch_psum()

            if _on():
                # global attention; tail emits LN3 stats (no Act ops) so the
                # act stream stays pure-exp until the sqrt/gelu tail
                _mark(nc, 'attn_g')
                for qb in range(SB):
                    def _t_op(sb=qb - 1):
                        op_block("g", sb)
                    def _t_stats(sb=qb - 1):
                        ln_stats(sb, f"B4{sb // 2}", sb % 2, stat_act=False)
                    attn_block("g", qb,
                               mid={1: _t_op, 3: _t_stats} if qb >= 1
                               else None)
                op_block("g", SB - 1)
                ln_stats(SB - 1, f"B4{(SB - 1) // 2}", (SB - 1) % 2, stat_act=False)
                _mark(nc, 'mlp_tail')
                ln_rstd((0,))
                ln_apply(0, 3)
                ln_rstd((1,))
                ln_apply(1, 3)
                ln_rstd((2,))
                ln_apply(2, 3)
                mlp_fc1(0)
                mlp_fc1(1)
                mlp_fc2(0)
                ln_rstd((3,))
                ln_apply(3, 3)
                mlp_fc1(2)
                mlp_fc2(1)
                mlp_fc1(3)
                mlp_fc2(2)
                mlp_fc2(3)

    nc.compile()
    return nc


def _prep_host_inputs(inputs):
    """Fold LN affine + 1/sqrt(hd) into weights, prescale, transpose, cast."""
    import ml_dtypes
    bf = ml_dtypes.bfloat16
    f8 = ml_dtypes.float8_e4m3
    f32 = np.float32

    def fold(W, b_proj, lw, lb):
        W_eff = (W * lw[None, :]).astype(f32)
        b_eff = (W @ lb + b_proj).astype(f32)
        return W_eff, b_eff

    wl, bl = fold(inputs["Wqkv_l"], inputs["bqkv_l"], inputs["ln1_w"], inputs["ln1_b"])
    wg, bg = fold(inputs["Wqkv_g"], inputs["bqkv_g"], inputs["ln2_w"], inputs["ln2_b"])
    qs = 1.0 / math.sqrt(HD)
    for w, b in ((wl, bl), (wg, bg)):
        w[:D] *= qs * S_Q
        b[:D] *= qs * S_Q
        w[D:2 * D] *= S_K
        b[D:2 * D] *= S_K
        w[2 * D:] *= S_V
        b[2 * D:] *= S_V
    w1, b1 = fold(inputs["W1"], inputs["b1"], inputs["ln3_w"], inputs["ln3_b"])

    # Additive band masks for the 2-slice local scores, stored transposed
    # ([q_local, k_local]) as the lhsT of a mask+identity matmul into the
    # score PSUM (scaled domain: -100 * S_Q*S_K kills the exp exactly).
    NEG = -100.0 * S_Q * S_K
    i = np.arange(128)
    ql = i[:, None]
    kl = i[None, :]
    masksadd = np.full((6, 128, 128), NEG, f32)

    def band(delta, extra=None):
        m = np.abs(kl + delta - ql) < BAND
        if extra is not None:
            m &= extra
        return np.where(m, 0.0, NEG)

    masksadd[0] = band(0, kl < 64)       # qt=0 slice0 (unshifted, k<64)
    masksadd[1] = band(64)               # qt=0 slice1 (o=64)
    masksadd[2] = band(-64)              # interior slice0 (o=128qt-64)
    masksadd[3] = band(64)               # interior slice1 (o=128qt+64)
    masksadd[4] = band(-64, kl < 64)     # qt=15 slice0 (o=1856, k<1920)
    masksadd[5] = band(0)                # qt=15 slice1 (o=1920)

    shared = {
        "wqkvT8_l": np.ascontiguousarray(wl.T).astype(f8),
        "wqkvT8_g": np.ascontiguousarray(wg.T).astype(f8),
        "bqk_l_r1": bl[:2 * D].reshape(1, -1).astype(bf),
        "bqk_g_r1": bg[:2 * D].reshape(1, -1).astype(bf),
        "bv_l_r1": bl[2 * D:].reshape(1, -1).astype(bf),
        "bv_g_r1": bg[2 * D:].reshape(1, -1).astype(bf),
        "woT8_l": np.ascontiguousarray(inputs["Wo_l"].T * S_O).astype(f8),
        "woT8_g": np.ascontiguousarray(inputs["Wo_g"].T * S_O).astype(f8),
        "bo_l_r1": (inputs["bo_l"].reshape(1, D) * ALPHA).astype(bf),
        "bo_g_r1": (inputs["bo_g"].reshape(1, D) * ALPHA).astype(bf),
        "w1T8": np.ascontiguousarray(w1.T * S_1).astype(f8),
        "b1": b1,
        "w2T8": np.ascontiguousarray(inputs["W2"].T * S_2).astype(f8),
        "b2_r1": (inputs["b2"].reshape(1, D) * ALPHA).astype(bf),
        "masksadd": masksadd.astype(bf),
    }
    return shared


_NC_CACHE = {}


def _get_nc(use_op_bias=False, use_qkv_bias=False, b1_nonzero=False):
    key = (use_op_bias, use_qkv_bias, b1_nonzero)
    if key not in _NC_CACHE:
        _NC_CACHE[key] = build(use_op_bias=use_op_bias,
                               use_qkv_bias=use_qkv_bias,
                               b1_nonzero=b1_nonzero)
    return _NC_CACHE[key]


def make_in_maps(inputs):
    import ml_dtypes
    shared = _prep_host_inputs(inputs)
    x = inputs["x"].astype(np.float32)
    in_maps = []
    for b in range(B):
        m = dict(shared)
        m["xTbf"] = np.ascontiguousarray(x[b].T * ALPHA).astype(ml_dtypes.bfloat16)
        in_maps.append(m)
    return in_maps


def kernel(**inputs):
    inputs = {k: np.asarray(v) for k, v in inputs.items()}
    use_op_bias = bool(
        np.any(inputs["bo_l"]) or np.any(inputs["bo_g"]) or np.any(inputs["b2"]))
    use_qkv_bias = bool(
        np.any(inputs["bqkv_l"]) or np.any(inputs["bqkv_g"])
        or np.any(inputs["Wqkv_l"] @ inputs["ln1_b"])
        or np.any(inputs["Wqkv_g"] @ inputs["ln2_b"]))
    b1_nonzero = bool(np.any(inputs["b1"]) or np.any(inputs["W1"] @ inputs["ln3_b"]))
    nc = _get_nc(use_op_bias=use_op_bias, use_qkv_bias=use_qkv_bias,
                 b1_nonzero=b1_nonzero)
    in_maps = make_in_maps(inputs)
    res = bass_utils.run_bass_kernel_spmd(nc, in_maps, core_ids=list(range(B)))
    out = np.stack([r["outT"].T for r in res.results], axis=0)
    return (out * (1.0 / ALPHA)).astype(np.float32)


if __name__ == "__main__":
    build()
    print("built ok")



# revision 74
# speedup vs baseline: 1.0159x; 1.0159x over previous
"""Trainium2 Bass kernel for nn_Block (LN -> local MHA -> LN -> global MHA -> LN -> MLP).

Sharding: pure data parallel, batch 8 across 8 cores (one batch element per
core), no collectives. All compute is feature-major ([D, S] transposed).

v4: on top of the fp8e4 DoubleRow / ALPHA-scaled bf16 residual design (see
scale ladder below), the schedule is tuned against the TimelineSim cost
model (297.4us -> 282.8us):

  - act-table hygiene: an explicit LoadActFuncSet(6) pins
    natural_log_exp_and_others (exp+ln+copy) at kernel start and rstd is
    computed as exp(-0.5*ln(var+eps)) on the Act engine, so the only act
    table switch left is the one into the Gelu set at the MLP tail
    (5 loads total vs 21).
  - the local-attention AV/den psum moved to its own bank pair ("avden"),
    decoupling the scores ring (PE->exp) from the normalize ring
    (AV -> recip/mul on DVE); local wave cadence no longer carries the
    DVE normalize latency.
  - out-proj residuals fold x into the psum via an identity matmul, making
    the writeback a pure drain that rotates across Act/DVE per phase
    (DRAIN_SEQ), like the qk/v projection drains; the xc8 quantize
    (SBUF-only) rotates across DVE/GPSIMD per LN layer (XC8_ENG). GPSIMD
    has no PSUM port, so only SBUF->SBUF sites may use it.
  - qkT/vnat are shared between the local and global layers (the global
    projections overwrite each s-block region only after the last
    local-attention read), halving their SBUF footprint.
  - input DMA issue order matches consumption order, and the first x
    s-block transfers in dt halves so LN1 stats start earlier.

Scale ladder (unchanged from v3):
  residual x' = ALPHA * x           (bf16; LN scale-invariant w/ eps' = eps*ALPHA^2)
  wq' = S_Q*Wq_eff, wk' = S_K*Wk, wv' = S_V*Wv  (fp8; xc8 = LN(x) true scale)
  scores psum = S_Q*S_K * s_true    -> exp(scale=1/(S_Q*S_K)) -> pt fp8 (true)
  V drains: v8 = S_V * v_true; den-ones = S_V/2 -> attnT = 2*attn_true (fp8)
  wo' = S_O*Wo with 2*S_O = ALPHA   -> out-proj psum = ALPHA*(Wo@attn)
  fc1 psum = S_1*h -> Gelu(scale=1/S_1) -> gT fp8 true; w2' = ALPHA*W2
"""

import math
import os
from contextlib import ExitStack

import numpy as np

import concourse.bacc as bacc
import concourse.bass as bass
import concourse.mybir as mybir
import concourse.tile as tile
from concourse import bass_utils

F32 = mybir.dt.float32
BF16 = mybir.dt.bfloat16
F8 = mybir.dt.float8e4
AF = mybir.ActivationFunctionType
ALU = mybir.AluOpType
DR = mybir.MatmulPerfMode.DoubleRow

NH = 4
BAND = 6
D = 512
B, S = 8, 2048
HD = 128
DT = D // 128
ET2 = (2 * D) // 128
SB = S // 512
ST = S // 128
EPS = 1e-5

ALPHA = 128.0
S_Q = 512.0
S_K = 64.0
S_V = 64.0
S_O = 64.0             # 2*S_O == ALPHA (attnT carries 2*attn via den-ones=S_V/2)
S_1 = 64.0
S_2 = ALPHA
EPS_EFF = EPS * ALPHA * ALPHA

_PHASE = {"n": 0}
MARKS = []


def _mark(nc, label):
    MARKS.append((label, nc.get_next_instruction_name()))


def _on():
    _PHASE["n"] += 1
    return _PHASE["n"] <= int(os.environ.get("K_STOP", "99"))


# Engine assignment for tunable elementwise sites: "v" = DVE, "g" = GPSIMD/Pool
# (GPSIMD has no PSUM port: only SBUF->SBUF sites may use "g".)
ENG = {
    "m2": "g",
    "unscale": "v",
}

# Per-dt engine for the xc8 quantize (SBUF->SBUF), keyed by LN layer.
XC8_ENG = {
    1: ("v", "g", "g", "g"),
    2: ("v", "g", "g", "v"),
    3: ("v", "v", "g", "g"),
}
# fc2 residual: accumulate x into the psum on PE (identity matmul), then
# drain on the otherwise-idle tail Act engine.


# PSUM->SBUF drain engine rotation per site ("a"=Act, "v"=DVE).
DRAIN_SEQ = {
    "qk_l": ("a", "a", "v"),
    "qk_g": ("a", "a", "v"),
    "v_l": ("a", "v"),
    "v_g": ("a", "v"),
    "op_l": ("a", "a", "v"),
    "op_g": ("v",),
    "fc2": ("v", "a"),
}
_DRAIN_CTR = {}


def _eng(nc, key):
    return nc.gpsimd if ENG[key] == "g" else nc.vector


def _drain(nc, dst, src_ap, site):
    seq = DRAIN_SEQ.get(site, ("v",))
    c = _DRAIN_CTR.get(site, 0)
    _DRAIN_CTR[site] = c + 1
    e = seq[c % len(seq)]
    if e == "a":
        nc.scalar.activation(dst, src_ap, AF.Copy)
    else:
        nc.vector.tensor_copy(dst, src_ap)


def build(use_op_bias=False, use_qkv_bias=False, b1_nonzero=False):
    _PHASE["n"] = 0
    MARKS.clear()
    _DRAIN_CTR.clear()
    nc = bacc.Bacc(trn_type="TRN2", target_bir_lowering=False, debug=False)
    drams = {}

    def din(name, shape, dtype, kind="ExternalInput"):
        drams[name] = nc.dram_tensor(name, shape, dtype, kind=kind)

    din("xTbf", [D, S], BF16)
    din("wqkvT8_l", [D, 3 * D], F8)
    din("wqkvT8_g", [D, 3 * D], F8)
    din("bqk_l_r1", [1, 2 * D], BF16)
    din("bqk_g_r1", [1, 2 * D], BF16)
    din("bv_l_r1", [1, D], BF16)
    din("bv_g_r1", [1, D], BF16)
    din("woT8_l", [D, D], F8)
    din("woT8_g", [D, D], F8)
    din("bo_l_r1", [1, D], BF16)
    din("bo_g_r1", [1, D], BF16)
    din("w1T8", [D, 2 * D], F8)
    din("b1", [2 * D], F32)
    din("w2T8", [2 * D, D], F8)
    din("b2_r1", [1, D], BF16)
    din("masksadd", [6, 128, 128], BF16)
    din("outT", [D, S], F32, kind="ExternalOutput")

    with tile.TileContext(nc) as tc:
        with ExitStack() as top:
            cpool = top.enter_context(tc.tile_pool(name="consts", bufs=1))
            ones_bf = cpool.tile([128, 128], BF16, tag="ones")
            nc.vector.memset(ones_bf, 1.0)           # LN stats matmul
            onesd_bf = cpool.tile([128, 128], BF16, tag="onesd")
            nc.vector.memset(onesd_bf, S_V / 2.0)    # local den (bf16 pt)
            ones8_2 = cpool.tile([128, 2, 128], F8, tag="ones8")
            nc.vector.memset(ones8_2, S_V / 2.0)     # global den (fp8 DR)
            ones_row = cpool.tile([1, 512], BF16, tag="onesr")
            nc.vector.memset(ones_row, 1.0)
            ones_col = cpool.tile([1, 128], BF16, tag="onesc")
            nc.vector.memset(ones_col, 1.0)
            _li = mybir.InstLoadActFuncSet(
                name=nc.get_next_instruction_name(), ins=[], outs=[],
                act_func_set_id=6)
            nc.scalar.add_instruction(_li)
            from concourse.masks import make_identity
            ident_bf = cpool.tile([128, 128], BF16, tag="ident")
            make_identity(nc, ident_bf)
            hid = top.enter_context(tc.tile_pool(name="hid", bufs=1))
            x = hid.tile([128, DT, S], BF16, tag="x")
            xbf_d = drams["xTbf"].ap().rearrange("(dt p) s -> p dt s", p=128)
            masks_sb = cpool.tile([128, 6, 128], BF16, tag="masks")

            wpool = top.enter_context(tc.tile_pool(name="weights", bufs=1))
            w8 = {}
            wo8 = {}
            bo_sb = {}
            bqk_r1 = {}
            bv_r1 = {}
            for wh in ("l", "g"):
                w8[wh] = wpool.tile([128, DT, 12 * 128], F8,
                                    tag=f"wqkv_{wh}", name=f"wqkv_{wh}")
                wo8[wh] = wpool.tile([128, NH, DT * 128], F8,
                                     tag=f"wo_{wh}", name=f"wo_{wh}")
                bo_sb[wh] = wpool.tile([1, 512], BF16, tag=f"bo_{wh}",
                                       name=f"bo_{wh}")
                if use_qkv_bias:
                    bqk_r1[wh] = wpool.tile([1, 1024], BF16,
                                            tag=f"bqk_{wh}", name=f"bqk_{wh}")
                    bv_r1[wh] = wpool.tile([1, 512], BF16, tag=f"bv_{wh}",
                                           name=f"bv_{wh}")
                else:
                    bqk_r1[wh] = bv_r1[wh] = None
            w18 = wpool.tile([128, DT, ET2 * 128], F8, tag="w1")
            w28 = wpool.tile([128, ET2, DT * 128], F8, tag="w2")
            b1_sb = wpool.tile([128, ET2], F32, tag="b1")
            b2_sb = wpool.tile([1, 512], BF16, tag="b2")

            # DMA issue order = consumption order: x(0), local weights, masks
            # (wave 0), remaining x, then the global/MLP weights.
            nc.sync.dma_start(x[:, 0:2, 0:512], xbf_d[:, 0:2, 0:512])
            nc.sync.dma_start(x[:, 2:4, 0:512], xbf_d[:, 2:4, 0:512])
            nc.sync.dma_start(w8["l"], drams["wqkvT8_l"].ap().rearrange(
                "(dt p) e -> p dt e", p=128))
            nc.sync.dma_start(masks_sb,
                              drams["masksadd"].ap().rearrange("m p j -> p m j"))
            for sb in range(1, SB):
                ssl = slice(sb * 512, (sb + 1) * 512)
                nc.sync.dma_start(x[:, :, ssl], xbf_d[:, :, ssl])
            nc.sync.dma_start(wo8["l"], drams["woT8_l"].ap().rearrange(
                "(h p) d -> p h d", p=128))
            nc.sync.dma_start(bo_sb["l"], drams["bo_l_r1"].ap())
            nc.sync.dma_start(w8["g"], drams["wqkvT8_g"].ap().rearrange(
                "(dt p) e -> p dt e", p=128))
            nc.sync.dma_start(wo8["g"], drams["woT8_g"].ap().rearrange(
                "(h p) d -> p h d", p=128))
            nc.sync.dma_start(bo_sb["g"], drams["bo_g_r1"].ap())
            if use_qkv_bias:
                for wh in ("l", "g"):
                    nc.sync.dma_start(bqk_r1[wh], drams[f"bqk_{wh}_r1"].ap())
                    nc.sync.dma_start(bv_r1[wh], drams[f"bv_{wh}_r1"].ap())
            nc.sync.dma_start(w18, drams["w1T8"].ap().rearrange(
                "(dt p) e -> p dt e", p=128))
            nc.sync.dma_start(w28, drams["w2T8"].ap().rearrange(
                "(e p) d -> p e d", p=128))
            nc.sync.dma_start(b1_sb, drams["b1"].ap().rearrange(
                "(e p) -> p e", p=128))
            nc.sync.dma_start(b2_sb, drams["b2_r1"].ap())

            act = top.enter_context(tc.tile_pool(name="act", bufs=1))
            xc8 = act.tile([128, DT, S], F8, tag="xc8")       # shared all layers
            # qkT / vnat are shared between the local and global layers: the
            # global projections overwrite each s-block region only after the
            # last local-attention read of it (subtile deps order the writes).
            qkT_sh = act.tile([128, 2 * NH, S], BF16, tag="qkT", name="qkT")
            qkT = {"l": qkT_sh, "g": qkT_sh}
            vnat_sh = act.tile([128, ST + 1, 512], F8, tag="vnat",
                               name="vnat")
            vnat = {"l": vnat_sh, "g": vnat_sh}
            attnT = act.tile([128, NH, S], F8, tag="attnT")   # shared l/g

            sbw = top.enter_context(tc.tile_pool(name="sbw", bufs=1))
            psA_stack = ExitStack()
            P = {"psum": psA_stack.enter_context(
                tc.tile_pool(name="psumA", bufs=1, space="PSUM"))}
            PH = {"bufs": 3}

            def switch_psum():
                """Close the local-phase psum pool (ps2 x3 + avden) and open
                the global-phase pool (big x1 + ps2 x1 + avden x1)."""
                psA_stack.close()
                P["psum"] = top.enter_context(
                    tc.tile_pool(name="psumB", bufs=1, space="PSUM"))
                PH["bufs"] = 1

            outT_d = drams["outT"].ap().rearrange("(dt p) s -> p dt s", p=128)

            # ---------------- per-s-block emitters ----------------

            ln_state = {}
            vpe_batches = {}

            def ln_stats(sb, bid, slot, ptag="ps2", pbufs=None,
                         stat_act=True, sq_eng="v"):
                """LN stats of residual x for one s-block.  The var+eps lands
                in slot `slot` of batch tile `bid` so a whole batch can be
                rstd'ed by a single sqrt instruction later."""
                ssl = slice(sb * 512, (sb + 1) * 512)
                sq = sbw.tile([128, DT, 512], BF16, tag="sq", bufs=1)
                if sq_eng == "a":
                    nc.scalar.activation(sq, x[:, :, ssl], AF.Square)
                else:
                    se = nc.vector if sq_eng == "v" else nc.gpsimd
                    se.tensor_mul(sq[:, 0:2, :], x[:, 0:2, ssl],
                                  x[:, 0:2, ssl])
                    se.tensor_mul(sq[:, 2:4, :], x[:, 2:4, ssl],
                                  x[:, 2:4, ssl])
                ps = P["psum"].tile([128, 2, 512], F32, tag=ptag,
                                    bufs=pbufs or PH["bufs"])
                for dt in range(DT):
                    nc.tensor.matmul(ps[:, 0, :], ones_bf, x[:, dt, ssl],
                                     start=(dt == 0), stop=(dt == DT - 1))
                    nc.tensor.matmul(ps[:, 1, :], ones_bf, sq[:, dt, :],
                                     start=(dt == 0), stop=(dt == DT - 1))
                if bid not in vpe_batches:
                    vpeb_t = sbw.tile([128, 2, 512], BF16, tag="vpeb",
                                      bufs=2, name=f"vpeb_{bid}")
                    vpe_batches[bid] = vpeb_t
                vpe = vpe_batches[bid][:, slot, :]
                meanb = sbw.tile([128, 512], BF16, tag="meanb", bufs=2)
                m2 = sbw.tile([128, 512], BF16, tag="m2", bufs=1)
                xcb = sbw.tile([128, DT, 512], BF16, tag="xcb", bufs=4)
                if stat_act:
                    nc.scalar.activation(meanb, ps[:, 0, :], AF.Copy,
                                         scale=1.0 / D)
                    nc.scalar.activation(vpe, ps[:, 1, :], AF.Copy,
                                         scale=1.0 / D, bias=EPS_EFF)
                else:
                    nc.vector.tensor_scalar(meanb, ps[:, 0, :], 1.0 / D, None,
                                            ALU.mult)
                    nc.vector.tensor_scalar(vpe, ps[:, 1, :], 1.0 / D,
                                            EPS_EFF, ALU.mult, ALU.add)
                _eng(nc, "m2").tensor_mul(m2, meanb, meanb)
                nc.vector.tensor_sub(vpe, vpe, m2)
                for dt in range(DT):
                    nc.vector.tensor_sub(xcb[:, dt, :], x[:, dt, ssl], meanb)
                ln_state[sb] = [bid, slot, xcb, None]

            def ln_rstd(sbs, recip=True):
                """rstd = exp(-0.5*ln(var+eps)) on the Act engine.  Both Ln
                and Exp live in act-func-set 6 (natural_log_exp_and_others),
                the set explicitly loaded at kernel start, so no act-table
                reloads happen no matter how the scheduler interleaves."""
                bid = ln_state[sbs[0]][0]
                slots = [ln_state[sb][1] for sb in sbs]
                lo, hi = min(slots), max(slots) + 1
                vpeb = vpe_batches[bid]
                lnv = sbw.tile([128, 2, 512], F32, tag="lnv", bufs=2)
                nc.scalar.activation(lnv[:, lo:hi, :], vpeb[:, lo:hi, :],
                                     AF.Ln)
                rstdb = sbw.tile([128, 2, 512], BF16, tag="rstdb", bufs=2)
                nc.scalar.activation(rstdb[:, lo:hi, :], lnv[:, lo:hi, :],
                                     AF.Exp, scale=-0.5)
                for sb in sbs:
                    ln_state[sb][3] = rstdb

            def ln_apply(sb, layer):
                """xc8 = xcb * rstd for one s-block (SBUF only: DVE/Pool)."""
                ssl = slice(sb * 512, (sb + 1) * 512)
                bid, slot, xcb, rstdb = ln_state.pop(sb)
                rstd = rstdb[:, slot, :]
                engs = XC8_ENG[layer]
                for dt in range(DT):
                    e = nc.vector if engs[dt] == "v" else nc.gpsimd
                    e.tensor_mul(xc8[:, dt, ssl], xcb[:, dt, :], rstd)

            # local V chunk starts: shifted grid so each q-tile's band is
            # covered by two adjacent chunks (DoubleRow-able)
            VCH = [0] + [128 * j - 64 for j in range(1, ST)] + [S - 128]
            VCH_SB = [[j for j in range(ST + 1)
                       if VCH[j] + 128 <= 512 * (sb + 1)
                       and (sb == 0 or VCH[j] + 128 > 512 * sb)]
                      for sb in range(SB)]

            def proj_v(sb, wh, ptag="ps2", pbufs=3):
                """V in natural (k-major) layout; xc8 chunk stationary."""
                w = w8[wh]
                chunks = (VCH_SB[sb] if wh == "l"
                          else list(range(4 * sb, 4 * sb + 4)))
                starts = {j: (VCH[j] if wh == "l" else j * 128)
                          for j in chunks}
                for p0 in range(0, len(chunks), 2):
                    pair = chunks[p0:p0 + 2]
                    ps = P["psum"].tile([128, 2, 512], F32, tag=ptag,
                                    bufs=pbufs or PH["bufs"])
                    for i, j in enumerate(pair):
                        csl = slice(starts[j], starts[j] + 128)
                        for dtp in range(0, DT, 2):
                            nc.tensor.matmul(
                                ps[:, i, :], xc8[:, dtp:dtp + 2, csl],
                                w[:, dtp:dtp + 2, 1024:1536],
                                start=(dtp == 0),
                                stop=(dtp == DT - 2 and bv_r1[wh] is None),
                                perf_mode=DR)
                        if bv_r1[wh] is not None:
                            nc.tensor.matmul(ps[:, i, :], ones_col, bv_r1[wh],
                                             start=False, stop=True)
                    if len(pair) == 2:
                        _drain(nc, vnat[wh][:, pair[0]:pair[0] + 2, :], ps, f"v_{wh}")
                    else:
                        _drain(nc, vnat[wh][:, pair[0], :], ps[:, 0, :], f"v_{wh}")

            def proj_qk(sb, wh, ptag="ps2", pbufs=3):
                w = w8[wh]
                ssl = slice(sb * 512, (sb + 1) * 512)
                for et0 in (4, 6, 0, 2):  # k heads first, then q
                    ps = P["psum"].tile([128, 2, 512], F32, tag=ptag,
                                    bufs=pbufs or PH["bufs"])
                    for i in range(2):
                        et = et0 + i
                        for dtp in range(0, DT, 2):
                            nc.tensor.matmul(
                                ps[:, i, :],
                                w[:, dtp:dtp + 2, et * 128:(et + 1) * 128],
                                xc8[:, dtp:dtp + 2, ssl],
                                start=(dtp == 0),
                                stop=(dtp == DT - 2 and bqk_r1[wh] is None),
                                perf_mode=DR)
                        if bqk_r1[wh] is not None:
                            nc.tensor.matmul(
                                ps[:, i, :],
                                bqk_r1[wh][:1, et * 128:(et + 1) * 128],
                                ones_row, start=False, stop=True)
                    _drain(nc, qkT[wh][:, et0:et0 + 2, ssl], ps, f"qk_{wh}")

            _attn_state = {"pre": {}}

            def attn_pre(wh, qb, h, ktps):
                """Head-start: scores+exp only for the given kt pairs; the
                pt tiles are stashed and consumed by the resume pass."""
                qk = qkT[wh]
                qsl = slice(qb * 512, (qb + 1) * 512)
                for ktp in ktps:
                    ps = P["psum"].tile([128, 2, 512], F32, tag="ps2",
                                        bufs=PH["bufs"])
                    for i in range(2):
                        kt = ktp + i
                        nc.tensor.matmul(
                            ps[:, i, :],
                            qk[:, NH + h, kt * 128:(kt + 1) * 128],
                            qk[:, h, qsl], start=True, stop=True)
                    pt = sbw.tile([128, 2, 512], F8, tag="pt", bufs=16)
                    nc.scalar.activation(pt, ps, AF.Exp,
                                         scale=1.0 / (S_Q * S_K))
                    _attn_state["pre"][(wh, qb, h, ktp)] = pt

            # ktp pairs per (qb, h), grouped so a 4-bank psum tile gives
            # one 2048-wide exp for two pairs (phase-B pools only).
            ATTN_GROUPS = (("b", (0, 2)), ("s", (4,)), ("b", (6, 8)),
                           ("s", (10,)), ("b", (12, 14)))

            def attn_block(wh, qb, mid=None):
                qk = qkT[wh]
                vn = vnat[wh]
                pre = _attn_state["pre"]
                qsl = slice(qb * 512, (qb + 1) * 512)
                for h in range(NH):
                    if mid is not None and h in mid:
                        mid[h]()
                    popd = P["psum"].tile([128, 2, 512], F32, tag="avden",
                                          bufs=1)
                    ndone = [0]

                    def avden_pair(ktp, pt_ap):
                        ndone[0] += 1
                        nc.tensor.matmul(
                            popd[:, 0, :],
                            vn[:, ktp:ktp + 2, h * 128:(h + 1) * 128],
                            pt_ap, start=(ndone[0] == 1),
                            stop=(ndone[0] == ST // 2), perf_mode=DR)
                        nc.tensor.matmul(
                            popd[:, 1, :], ones8_2, pt_ap,
                            start=(ndone[0] == 1), stop=(ndone[0] == ST // 2),
                            perf_mode=DR)

                    def score_pair_into(ps_rows, k):
                        for j in range(2):
                            nc.tensor.matmul(
                                ps_rows[:, j, :],
                                qk[:, NH + h, (k + j) * 128:(k + j + 2) * 128
                                   - 128],
                                qk[:, h, qsl], start=True, stop=True)

                    for kind, ktps in ATTN_GROUPS:
                        stashed = [k for k in ktps
                                   if (wh, qb, h, k) in pre]
                        missing = [k for k in ktps if k not in stashed]
                        for k in stashed:
                            avden_pair(k, pre.pop((wh, qb, h, k)))
                        if not missing:
                            continue
                        if kind == "b" and len(missing) == 2:
                            ps = P["psum"].tile([128, 4, 512], F32,
                                                tag="big", bufs=1)
                            score_pair_into(ps[:, 0:2, :], missing[0])
                            score_pair_into(ps[:, 2:4, :], missing[1])
                            pt4 = sbw.tile([128, 4, 512], F8, tag="pt4",
                                           bufs=2)
                            nc.scalar.activation(pt4, ps, AF.Exp,
                                                 scale=1.0 / (S_Q * S_K))
                            avden_pair(missing[0], pt4[:, 0:2, :])
                            avden_pair(missing[1], pt4[:, 2:4, :])
                        else:
                            for k in missing:
                                ps = P["psum"].tile([128, 2, 512], F32,
                                                    tag="ps2",
                                                    bufs=PH["bufs"])
                                score_pair_into(ps, k)
                                pt = sbw.tile([128, 2, 512], F8, tag="pt",
                                              bufs=16)
                                nc.scalar.activation(
                                    pt, ps, AF.Exp, scale=1.0 / (S_Q * S_K))
                                avden_pair(k, pt)
                    rden = sbw.tile([128, 512], F32, tag="rden", bufs=2)
                    nc.vector.reciprocal(rden, popd[:, 1, :])
                    nc.vector.tensor_mul(attnT[:, h, qb * 512:(qb + 1) * 512],
                                         popd[:, 0, :], rden)

            def attn_local_factory():
                """Local attention, qt-major with all heads batched.
                Scores + additive band masks accumulate in one [128,4,2,128]
                PSUM quad; one exp per q-tile; fp8 DoubleRow AV/den on the
                shifted V grid; per-qt normalize. Returns step(w) emitting
                one skewed pipeline wave; call w = 0..ST+1."""
                qk = qkT["l"]
                vn = vnat["l"]
                sc = {}
                pts = {}
                pops = {}

                def emit_scores(qt):
                    ps = P["psum"].tile([128, NH, 2, 128], F32, tag="ps2",
                                        bufs=3)
                    sc[qt] = ps
                    # mask class: 0 first tile, 1 interior, 2 last
                    cls = 0 if qt == 0 else (2 if qt == ST - 1 else 1)
                    qsl = slice(qt * 128, (qt + 1) * 128)
                    for h in range(NH):
                        for i in range(2):
                            o = VCH[qt + i]
                            nc.tensor.matmul(
                                ps[:, h, i, :], qk[:, NH + h, o:o + 128],
                                qk[:, h, qsl], start=True, stop=False)
                            nc.tensor.matmul(
                                ps[:, h, i, :],
                                masks_sb[:, 2 * cls + i, :], ident_bf,
                                start=False, stop=True)

                def emit_exp(qt):
                    pt = sbw.tile([128, NH, 2, 128], F8, tag="ptl", bufs=3)
                    pts[qt] = pt
                    nc.scalar.activation(pt, sc[qt], AF.Exp,
                                         scale=1.0 / (S_Q * S_K))
                    del sc[qt]

                def emit_avden(qt):
                    popd = P["psum"].tile([128, 2, NH, 128], F32,
                                          tag="avden", bufs=1)
                    pops[qt] = popd
                    pt = pts[qt]
                    for h in range(NH):
                        nc.tensor.matmul(
                            popd[:, 0, h, :],
                            vn[:, qt:qt + 2, h * 128:(h + 1) * 128],
                            pt[:, h, :, :], start=True, stop=True,
                            perf_mode=DR)
                        nc.tensor.matmul(
                            popd[:, 1, h, :], ones8_2, pt[:, h, :, :],
                            start=True, stop=True, perf_mode=DR)
                    del pts[qt]

                def emit_norm(qt):
                    popd = pops.pop(qt)
                    qsl = slice(qt * 128, (qt + 1) * 128)
                    rden = sbw.tile([128, NH, 128], F32, tag="rden", bufs=2)
                    nc.vector.reciprocal(rden, popd[:, 1, :, :])
                    nc.vector.tensor_mul(attnT[:, :, qsl], popd[:, 0, :, :],
                                         rden)

                def step(w):
                    if w < ST:
                        emit_scores(w)
                    if 1 <= w <= ST:
                        emit_exp(w - 1)
                    if w >= 2:
                        emit_avden(w - 2)
                        emit_norm(w - 2)

                return step

            def op_block(wh, sb):
                """Out-proj + residual: x folded into the psum via an
                identity matmul; the writeback is then a pure drain that can
                rotate across engines."""
                ssl = slice(sb * 512, (sb + 1) * 512)
                for dtp in range(0, DT, 2):
                    ps = P["psum"].tile([128, 2, 512], F32, tag="ps2",
                                        bufs=PH["bufs"])
                    for i in range(2):
                        dt = dtp + i
                        for hp in range(0, NH, 2):
                            nc.tensor.matmul(
                                ps[:, i, :],
                                wo8[wh][:, hp:hp + 2, dt * 128:(dt + 1) * 128],
                                attnT[:, hp:hp + 2, ssl],
                                start=(hp == 0), stop=False,
                                perf_mode=DR)
                        if use_op_bias:
                            nc.tensor.matmul(
                                ps[:, i, :],
                                bo_sb[wh][:1, dt * 128:(dt + 1) * 128],
                                ones_row, start=False, stop=False)
                        nc.tensor.matmul(
                            ps[:, i, :], ident_bf, x[:, dt, ssl],
                            start=False, stop=True)
                    _drain(nc, x[:, dtp:dtp + 2, ssl], ps, f"op_{wh}")

            g_tiles = {}

            def mlp_fc1(sb):
                ssl = slice(sb * 512, (sb + 1) * 512)
                gT = sbw.tile([128, ET2, 512], F8, tag="gT", bufs=2)
                g_tiles[sb] = gT
                for e2q in range(0, ET2, 4):
                    ps = P["psum"].tile([128, 4, 512], F32, tag="big", bufs=1)
                    for i in range(4):
                        e2 = e2q + i
                        for dtp in range(0, DT, 2):
                            nc.tensor.matmul(
                                ps[:, i, :],
                                w18[:, dtp:dtp + 2, e2 * 128:(e2 + 1) * 128],
                                xc8[:, dtp:dtp + 2, ssl],
                                start=(dtp == 0), stop=(dtp == DT - 2),
                                perf_mode=DR)
                    if b1_nonzero:
                        for i in range(4):
                            nc.scalar.activation(
                                gT[:, e2q + i, :], ps[:, i, :], AF.Gelu,
                                bias=b1_sb[:, e2q + i:e2q + i + 1],
                                scale=1.0 / S_1)
                    else:
                        nc.scalar.activation(gT[:, e2q:e2q + 4, :], ps,
                                             AF.Gelu, scale=1.0 / S_1)

            def mlp_fc2(sb):
                ssl = slice(sb * 512, (sb + 1) * 512)
                gT = g_tiles.pop(sb)
                for dtp in range(0, DT, 2):
                    if dtp == 0:
                        ps = P["psum"].tile([128, 2, 512], F32, tag="ps2",
                                        bufs=PH["bufs"])
                    else:
                        ps = P["psum"].tile([128, 2, 512], F32, tag="ps2",
                                            bufs=PH["bufs"])
                    xo = sbw.tile([128, 2, 512], F32, tag="xout", bufs=2)
                    for i in range(2):
                        dt = dtp + i
                        for e2p in range(0, ET2, 2):
                            nc.tensor.matmul(
                                ps[:, i, :],
                                w28[:, e2p:e2p + 2, dt * 128:(dt + 1) * 128],
                                gT[:, e2p:e2p + 2, :],
                                start=(e2p == 0), stop=False,
                                perf_mode=DR)
                        if use_op_bias:
                            nc.tensor.matmul(
                                ps[:, i, :],
                                b2_sb[:1, dt * 128:(dt + 1) * 128],
                                ones_row, start=False, stop=False)
                        nc.tensor.matmul(
                            ps[:, i, :], ident_bf, x[:, dt, ssl],
                            start=False, stop=True)
                    _drain(nc, xo, ps, "fc2")
                    nc.sync.dma_start(outT_d[:, dtp:dtp + 2, ssl], xo)

            # ---------------- pipeline schedule ----------------

            if _on():
                step = attn_local_factory()
                def ln_inline(sb, bid, layer):
                    ln_stats(sb, bid, 0)
                    ln_rstd((sb,))
                    ln_apply(sb, layer)

                _mark(nc, 'prologue')
                ln_inline(0, "L10", 1)
                ln_inline(1, "L11", 1)
                ln_inline(2, "L12", 1)
                proj_qk(0, "l")
                proj_v(0, "l")
                _mark(nc, 'waves012')
                for w in (0, 1, 2):
                    step(w)
                ln_inline(3, "L13", 1)
                proj_qk(1, "l")
                proj_v(1, "l")
                for w in (3, 4, 5, 6):
                    step(w)
                proj_qk(2, "l")
                proj_v(2, "l")
                for w in (7, 8, 9):
                    step(w)
                op_block("l", 0)
                ln_inline(0, "L20", 2)
                step(10)
                proj_qk(3, "l")
                proj_v(3, "l")
                for w in (11, 12, 13):
                    step(w)
                op_block("l", 1)
                ln_inline(1, "L21", 2)
                proj_qk(0, "g")
                proj_v(0, "g")
                _mark(nc, 'waves14+')
                for w in (14, 15, 16):
                    step(w)
                op_block("l", 2)
                ln_inline(2, "L22", 2)
                proj_qk(1, "g")
                proj_v(1, "g")
                step(17)
                _mark(nc, 'op_l3')
                op_block("l", 3)
                ln_inline(3, "L23", 2)
                proj_qk(2, "g")
                proj_v(2, "g")
                _mark(nc, 'projg3')
                attn_pre("g", 0, 0, (0, 2, 4, 6))
                attn_pre("g", 0, 1, (0, 2))
                proj_qk(3, "g")
                proj_v(3, "g")
                switch_psum()

            if _on():
                # global attention; tail emits LN3 stats (no Act ops) so the
                # act stream stays pure-exp until the sqrt/gelu tail
                _mark(nc, 'attn_g')
                for qb in range(SB):
                    def _t_op(sb=qb - 1):
                        op_block("g", sb)
                    def _t_stats(sb=qb - 1):
                        ln_stats(sb, f"B4{sb // 2}", sb % 2, stat_act=False)
                    attn_block("g", qb,
                               mid={1: _t_op, 3: _t_stats} if qb >= 1
                               else None)
                op_block("g", SB - 1)
                ln_stats(SB - 1, f"B4{(SB - 1) // 2}", (SB - 1) % 2, stat_act=False)
                _mark(nc, 'mlp_tail')
                ln_rstd((0,))
                ln_apply(0, 3)
                ln_rstd((1,))
                ln_apply(1, 3)
                ln_rstd((2,))
                ln_apply(2, 3)
                mlp_fc1(0)
                mlp_fc1(1)
                mlp_fc2(0)
                ln_rstd((3,))
                ln_apply(3, 3)
                mlp_fc1(2)
                mlp_fc2(1)
                mlp_fc1(3)
                mlp_fc2(2)
                mlp_fc2(3)

    nc.compile()
    return nc


def _prep_host_inputs(inputs):
    """Fold LN affine + 1/sqrt(hd) into weights, prescale, transpose, cast."""
    import ml_dtypes
    bf = ml_dtypes.bfloat16
    f8 = ml_dtypes.float8_e4m3
    f32 = np.float32

    def fold(W, b_proj, lw, lb):
        W_eff = (W * lw[None, :]).astype(f32)
        b_eff = (W @ lb + b_proj).astype(f32)
        return W_eff, b_eff

    wl, bl = fold(inputs["Wqkv_l"], inputs["bqkv_l"], inputs["ln1_w"], inputs["ln1_b"])
    wg, bg = fold(inputs["Wqkv_g"], inputs["bqkv_g"], inputs["ln2_w"], inputs["ln2_b"])
    qs = 1.0 / math.sqrt(HD)
    for w, b in ((wl, bl), (wg, bg)):
        w[:D] *= qs * S_Q
        b[:D] *= qs * S_Q
        w[D:2 * D] *= S_K
        b[D:2 * D] *= S_K
        w[2 * D:] *= S_V
        b[2 * D:] *= S_V
    w1, b1 = fold(inputs["W1"], inputs["b1"], inputs["ln3_w"], inputs["ln3_b"])

    # Additive band masks for the 2-slice local scores, stored transposed
    # ([q_local, k_local]) as the lhsT of a mask+identity matmul into the
    # score PSUM (scaled domain: -100 * S_Q*S_K kills the exp exactly).
    NEG = -100.0 * S_Q * S_K
    i = np.arange(128)
    ql = i[:, None]
    kl = i[None, :]
    masksadd = np.full((6, 128, 128), NEG, f32)

    def band(delta, extra=None):
        m = np.abs(kl + delta - ql) < BAND
        if extra is not None:
            m &= extra
        return np.where(m, 0.0, NEG)

    masksadd[0] = band(0, kl < 64)       # qt=0 slice0 (unshifted, k<64)
    masksadd[1] = band(64)               # qt=0 slice1 (o=64)
    masksadd[2] = band(-64)              # interior slice0 (o=128qt-64)
    masksadd[3] = band(64)               # interior slice1 (o=128qt+64)
    masksadd[4] = band(-64, kl < 64)     # qt=15 slice0 (o=1856, k<1920)
    masksadd[5] = band(0)                # qt=15 slice1 (o=1920)

    shared = {
        "wqkvT8_l": np.ascontiguousarray(wl.T).astype(f8),
        "wqkvT8_g": np.ascontiguousarray(wg.T).astype(f8),
        "bqk_l_r1": bl[:2 * D].reshape(1, -1).astype(bf),
        "bqk_g_r1": bg[:2 * D].reshape(1, -1).astype(bf),
        "bv_l_r1": bl[2 * D:].reshape(1, -1).astype(bf),
        "bv_g_r1": bg[2 * D:].reshape(1, -1).astype(bf),
        "woT8_l": np.ascontiguousarray(inputs["Wo_l"].T * S_O).astype(f8),
        "woT8_g": np.ascontiguousarray(inputs["Wo_g"].T * S_O).astype(f8),
        "bo_l_r1": (inputs["bo_l"].reshape(1, D) * ALPHA).astype(bf),
        "bo_g_r1": (inputs["bo_g"].reshape(1, D) * ALPHA).astype(bf),
        "w1T8": np.ascontiguousarray(w1.T * S_1).astype(f8),
        "b1": b1,
        "w2T8": np.ascontiguousarray(inputs["W2"].T * S_2).astype(f8),
        "b2_r1": (inputs["b2"].reshape(1, D) * ALPHA).astype(bf),
        "masksadd": masksadd.astype(bf),
    }
    return shared


_NC_CACHE = {}


def _get_nc(use_op_bias=False, use_qkv_bias=False, b1_nonzero=False):
    key = (use_op_bias, use_qkv_bias, b1_nonzero)
    if key not in _NC_CACHE:
        _NC_CACHE[key] = build(use_op_bias=use_op_bias,
                               use_qkv_bias=use_qkv_bias,
                               b1_nonzero=b1_nonzero)
    return _NC_CACHE[key]


def make_in_maps(inputs):
    import ml_dtypes
    shared = _prep_host_inputs(inputs)
    x = inputs["x"].astype(np.float32)
    in_maps = []
    for b in range(B):
        m = dict(shared)
        m["xTbf"] = np.ascontiguousarray(x[b].T * ALPHA).astype(ml_dtypes.bfloat16)
        in_maps.append(m)
    return in_maps


def kernel(**inputs):
    inputs = {k: np.asarray(v) for k, v in inputs.items()}
    use_op_bias = bool(
        np.any(inputs["bo_l"]) or np.any(inputs["bo_g"]) or np.any(inputs["b2"]))
    use_qkv_bias = bool(
        np.any(inputs["bqkv_l"]) or np.any(inputs["bqkv_g"])
        or np.any(inputs["Wqkv_l"] @ inputs["ln1_b"])
        or np.any(inputs["Wqkv_g"] @ inputs["ln2_b"]))
    b1_nonzero = bool(np.any(inputs["b1"]) or np.any(inputs["W1"] @ inputs["ln3_b"]))
    nc = _get_nc(use_op_bias=use_op_bias, use_qkv_bias=use_qkv_bias,
                 b1_nonzero=b1_nonzero)
    in_maps = make_in_maps(inputs)
    res = bass_utils.run_bass_kernel_spmd(nc, in_maps, core_ids=list(range(B)))
    out = np.stack([r["outT"].T for r in res.results], axis=0)
    return (out * (1.0 / ALPHA)).astype(np.float32)


if __name__ == "__main__":
    build()
    print("built ok")

